# revision 4
# baseline (speedup 1.0000x reference)
"""Trainium2 Bass kernel for nn_AttnNet_50852412784797.

The module computes, per (b, s):
    scores = V . tanh(Wi@Ec_i + Wj@Ec_j);  alpha = softmax_j(scores)
    attn_i = sum_j alpha[i, j] * Ec[..., i, :]      # Ec indexed by i, NOT j
Because Ec is broadcast along the softmax-summed axis j and each softmax
row sums to 1, the output is exactly Ec reshaped to (B, S, 1, L*D); the
reference's only deviation from Ec is fp32 softmax-normalization noise
(~2e-7 relative, verified numerically against the reference).

The memory-roofline kernel is therefore pure data movement: shard Ec
data-parallel over the B*S rows across the 8 cores (per the sharding
hint) and copy each core's 256 KB shard DRAM->DRAM.

Per-core kernel structure (raw Bass, no Tile):
- The copy is split across the two HWDGE rings (SP/sync + Activation/
  scalar) so descriptor generation and completion receipts overlap; each
  ring's InstDMACopy fans out across all 16 SDMA engines, together
  saturating the ~358 GB/s per-core HBM bandwidth (measured transfer
  window ~1.1 us for 256 KB read+write).
- Each issuing engine waits on its DMA's completion semaphore (+16)
  before reaching the compiler-emitted epilogue, so the NEFF cannot
  complete before the output bytes have landed (engine DRAIN alone does
  not fence DMA receipts).
- The Bass-constructor preamble (const-AP memsets, per-engine register
  defaults, all-engine barrier) is dead code for this kernel -- nothing
  here touches SBUF constants or engine registers -- so it is stripped
  from the BIR, shortening every engine's stream. One 1-element SBUF
  memset is kept as the first body instruction: it re-initializes the
  const-0 AP the framework would have set up, and marks the body start
  for profiling.
Remaining NEFF time is dominated by the fixed compiler epilogue (each
engine resets its ~50-semaphore bank; the PE engine's chain is ~6 us).
"""

import numpy as np

_AXON_PATHS = [
    "/root/.axon_site",
    "/root/.axon_site/_ro/trn_rl_repo",
    "/root/.axon_site/_ro/pypackages",
    "/opt/trn_rl_repo",
]


def _import_concourse():
    try:
        import concourse.mybir as mybir
        from concourse import bass
        from concourse.bass_utils import run_bass_kernel_spmd
    except ImportError:
        import sys

        for p in _AXON_PATHS:
            if p not in sys.path:
                sys.path.append(p)
        import concourse.mybir as mybir
        from concourse import bass
        from concourse.bass_utils import run_bass_kernel_spmd
    return bass, mybir, run_bass_kernel_spmd


B, SLIDE, L, D = 4, 16, 128, 64
N_CORES = 8
ROWS = B * SLIDE                  # 64 (b, s) pairs
ROWS_PER_CORE = ROWS // N_CORES   # 8
ROW_ELEMS = L * D                 # 8192
_SYNC_ROWS = ROWS_PER_CORE // 2   # half per HWDGE ring

_NC_CACHE = None


def _strip_dead_preamble(nc, n_preamble):
    """Drop the constructor-emitted preamble this kernel never uses.

    The kernel's body is static DRAM->DRAM DMA + semaphore waits: it
    reads no engine registers (InstRegisterMove), no const APs
    (InstMemset), and needs no engine synchronization before the body
    (InstDrain/InstEventSemaphore all-engine barrier) because each DMA
    depends only on DRAM inputs that are resident before the NEFF
    starts. The body instructions are spliced directly after the DMA-
    table dummy Call, which must stay first.
    """
    bb0 = nc.m.functions[0].blocks[0]
    insts = bb0.instructions
    pre, body = insts[:n_preamble], insts[n_preamble:]
    kept = [
        ins
        for ins in pre[1:]
        if type(ins).__name__
        not in ("InstMemset", "InstDrain", "InstEventSemaphore", "InstRegisterMove")
    ]
    insts[:] = [pre[0]] + body + kept


def build_bass_kernel():
    """One SPMD program: copy this core's (8, 8192) f32 shard in -> out."""
    global _NC_CACHE
    if _NC_CACHE is not None:
        return _NC_CACHE
    try:
        nc = _build(strip=True)
    except Exception:
        nc = _build(strip=False)
    _NC_CACHE = nc
    return nc


def _build(strip):
    bass, mybir, _ = _import_concourse()

    nc = bass.Bass()
    n_preamble = len(nc.m.functions[0].blocks[0].instructions)
    x = nc.declare_dram_parameter(
        "x", [ROWS_PER_CORE, ROW_ELEMS], mybir.dt.float32, isOutput=False
    )
    y = nc.declare_dram_parameter(
        "y", [ROWS_PER_CORE, ROW_ELEMS], mybir.dt.float32, isOutput=True
    )

    # Body-start marker: re-init the framework's const-0 AP (1 SBUF elem).
    nc.gpsimd.memset(nc.const_aps.aps[(mybir.dt.float32, 0.0)], 0)

    s_sync = nc.ctx.enter_context(nc.semaphore("dma_sem_sync"))
    s_scal = nc.ctx.enter_context(nc.semaphore("dma_sem_scal"))
    h = _SYNC_ROWS
    nc.sync.dma_start(out=y[:h], in_=x[:h]).then_inc(s_sync, 16)
    nc.scalar.dma_start(out=y[h:], in_=x[h:]).then_inc(s_scal, 16)
    nc.sync.wait_ge(s_sync, 16)
    nc.scalar.wait_ge(s_scal, 16)

    if strip:
        _strip_dead_preamble(nc, n_preamble)
    return nc


def shard_inputs(Ec):
    flat = np.ascontiguousarray(np.asarray(Ec, dtype=np.float32)).reshape(
        ROWS, ROW_ELEMS
    )
    return [
        {"x": flat[i * ROWS_PER_CORE : (i + 1) * ROWS_PER_CORE]}
        for i in range(N_CORES)
    ]


def unshard_output(results):
    out = np.concatenate([results[i]["y"] for i in range(N_CORES)], axis=0)
    return out.reshape(B, SLIDE, 1, ROW_ELEMS)


def kernel(Ec, Wi, Wj, V):
    _, _, run_bass_kernel_spmd = _import_concourse()
    nc = build_bass_kernel()
    in_maps = shard_inputs(Ec)
    res = run_bass_kernel_spmd(nc, in_maps, list(range(N_CORES)))
    return unshard_output(res.results)
